# revision 8
# baseline (speedup 1.0000x reference)
"""CTC best-path decoder (beam_width=1) for Trainium2, 8 NeuronCores.

Heavy part (argmax + max over the 2500-class axis of [64,256,2500] softmax
probs) runs on device, data-parallel over the batch: each core gets 8
sequences = 2048 (b,t) rows = 16 tiles of [128, 2500], processed as 2
batches of 8 tiles living in one contiguous SBUF mega-tile each.

Per batch of 8 tiles:
  1. 8 streaming DMAs [128,2500] into the mega-tile (the only full-rate
     HBM traffic, ~57us/core total across both batches)
  2. 4 pair-wise DVE reduce_max over subchunks of 20 -> fmb[128,1000]
     (the only full-rate Vector-engine scans)
  3. 8 per-tile DVE max (top-8) on fmb slices -> row maxes
  4. one strided copy collects the 8 row maxes -> gm8[128,8]
  5. one batched DVE max_index(gm8, fmb) -> winning subchunk cfb per tile
  6. one GPSIMD ap_gather pulls, per 16-partition group, the 128 winning
     subchunks (8 tiles x 16 partners) -> gb[128, 128*20]
  7. one batched DVE max_index(gm8, gb) -> q; host maps within-subchunk
     offset w = q % 20, argmax = (cfb % 125)*20 + w
Host verifies x[row, argmax] == rowmax (numpy re-argmax fallback on any
mismatch), then computes scores = -sum(log(maxp+eps)) and the cheap
[64,256] CTC collapse in numpy. log is monotone so argmax(p) ==
argmax(log(p+eps)) and max(log(p+eps)) == log(max(p)+eps) exactly.
"""

import numpy as np

import concourse.bacc as bacc
import concourse.mybir as mybir
import concourse.tile as tile
from concourse.bass_utils import run_bass_kernel_spmd

EPS = 1e-7

B, T, C = 64, 256, 2500
NCORES = 8
BLOC = B // NCORES          # sequences per core
ROWS = BLOC * T             # 2048 (b,t) rows per core
P = 128
NTILES = ROWS // P          # 16
D = 20                      # subchunk width
NSUB = C // D               # 125 subchunks per row
BATCH = 8                   # tiles per mega-batch
NBATCH = NTILES // BATCH    # 2


def build():
    nc = bacc.Bacc("TRN2", target_bir_lowering=False, debug=False)

    x = nc.declare_dram_parameter("x", [ROWS, C], mybir.dt.float32, isOutput=False)
    out_val = nc.declare_dram_parameter(
        "out_val", [P, NTILES * 8], mybir.dt.float32, isOutput=True
    )
    out_cf = nc.declare_dram_parameter(
        "out_cf", [P, NBATCH * 8], mybir.dt.uint16, isOutput=True
    )
    out_q = nc.declare_dram_parameter(
        "out_q", [P, NBATCH * 8], mybir.dt.uint16, isOutput=True
    )

    x_tiles = x[:].rearrange("(n p) c -> n p c", p=P)        # [16,128,2500]

    with tile.TileContext(nc) as tc:
        with (
            tc.tile_pool(name="xp", bufs=2) as xp,
            tc.tile_pool(name="sp", bufs=2) as sp,
            tc.tile_pool(name="acc", bufs=1) as accp,
        ):
            gm_all = accp.tile([P, NTILES * 8], mybir.dt.float32)
            cf_all = accp.tile([P, NBATCH * 8], mybir.dt.uint16)
            q_all = accp.tile([P, NBATCH * 8], mybir.dt.uint16)

            for b in range(NBATCH):
                xtb = xp.tile([P, BATCH * C], mybir.dt.float32, tag="xtb")
                for k in range(BATCH):
                    nc.sync.dma_start(
                        out=xtb[:, k * C : (k + 1) * C], in_=x_tiles[b * BATCH + k]
                    )

                fmb = sp.tile([P, BATCH * NSUB], mybir.dt.float32, tag="fmb")
                for j in range(BATCH // 2):     # pair-wise reduces
                    nc.vector.reduce_max(
                        out=fmb[:, j * 2 * NSUB : (j + 1) * 2 * NSUB],
                        in_=xtb[:, j * 2 * C : (j + 1) * 2 * C].rearrange(
                            "p (k g) -> p k g", g=D
                        ),
                        axis=mybir.AxisListType.X,
                    )

                for k in range(BATCH):
                    t = b * BATCH + k
                    nc.vector.max(
                        out=gm_all[:, t * 8 : (t + 1) * 8],
                        in_=fmb[:, k * NSUB : (k + 1) * NSUB],
                    )

                gm8 = sp.tile([P, 8], mybir.dt.float32, tag="gm8")
                nc.vector.tensor_copy(
                    out=gm8[:],
                    in_=gm_all[:, b * BATCH * 8 : (b + 1) * BATCH * 8].rearrange(
                        "p (k e) -> p k e", e=8
                    )[:, :, 0],
                )

                cfb = cf_all[:, b * 8 : (b + 1) * 8]
                nc.vector.max_index(out=cfb, in_max=gm8[:], in_values=fmb[:])

                gb = sp.tile([P, 128 * D], mybir.dt.float32, tag="gb")
                nc.gpsimd.ap_gather(
                    out_ap=gb[:],
                    in_ap=xtb[:],
                    idxs_ap=cf_all[:, b * 8 : (b + 1) * 8].bitcast(mybir.dt.int16),
                    channels=P,
                    num_elems=BATCH * NSUB,
                    d=D,
                    num_idxs=128,
                )
                nc.vector.max_index(
                    out=q_all[:, b * 8 : (b + 1) * 8], in_max=gm8[:], in_values=gb[:]
                )

            nc.sync.dma_start(out=out_val[:], in_=gm_all[:])
            nc.sync.dma_start(out=out_cf[:], in_=cf_all[:])
            nc.sync.dma_start(out=out_q[:], in_=q_all[:])

    nc.finalize()
    return nc


def _run_device(x, trace=False):
    """x: [64,256,2500] f32 -> (best [64,256] int64, maxp [64,256] f32, results)"""
    shards = np.ascontiguousarray(x.reshape(NCORES, ROWS, C))
    nc = build()
    in_maps = [{"x": shards[i]} for i in range(NCORES)]
    res = run_bass_kernel_spmd(
        nc, in_maps, core_ids=list(range(NCORES)), trace=trace
    )
    best = np.empty((NCORES, ROWS), np.int64)
    maxp = np.empty((NCORES, ROWS), np.float32)
    for i in range(NCORES):
        gm = res.results[i]["out_val"][:, ::8]                  # [128,16]
        cf = res.results[i]["out_cf"].astype(np.int64)          # [128,16]
        q = res.results[i]["out_q"].astype(np.int64)            # [128,16]
        fi = (cf % NSUB) * D + (q % D)
        maxp[i] = gm.T.reshape(ROWS)
        best[i] = fi.T.reshape(ROWS)
    best = best.reshape(B * T)
    maxp = maxp.reshape(B * T)

    # Safety: the device argmax must reproduce the device max value; any row
    # where it doesn't (ties across subchunks, unexpected layout issue) is
    # recomputed exactly on host.
    x2d = x.reshape(B * T, C)
    bad = x2d[np.arange(B * T), best] != maxp
    if bad.any():
        best[bad] = np.argmax(x2d[bad], axis=1)
    return best.reshape(B, T), maxp.reshape(B, T), res


def _decode(best, maxp):
    """CTC collapse + scores, numpy, matching K.ctc_decode semantics."""
    blank = C - 1
    max_lp = np.log(maxp + np.float32(EPS)).astype(np.float32)
    scores = -np.sum(max_lp, axis=1, keepdims=True, dtype=np.float32)

    prev = np.concatenate(
        [np.full((B, 1), -1, dtype=best.dtype), best[:, :-1]], axis=1
    )
    keep = (best != blank) & (best != prev)
    pos = np.cumsum(keep.astype(np.int64), axis=1) - 1
    pos = np.where(keep, pos, T)
    out = np.full((B, T + 1), -1, dtype=np.int32)
    out[np.arange(B)[:, None], pos] = best.astype(np.int32)
    decoded = out[:, :T]
    return decoded, scores.astype(np.float32)


def kernel(inputs):
    x = np.ascontiguousarray(np.asarray(inputs, dtype=np.float32))
    assert x.shape == (B, T, C)
    best, maxp, _ = _run_device(x)
    return _decode(best, maxp)
